# revision 47
# baseline (speedup 1.0000x reference)
"""Multi-head causal attention (B=2, T=2048, D=1024, H=16) on 8 TRN2 cores.

Sharding: core = 4*b + g handles batch b, heads 4g..4g+3 (head/tensor
parallel).  Each core computes its 4 heads end-to-end plus a partial
out-projection; the host sums the 4 partials per batch and adds biases
that commute with the linear ops (b_out, and b_v folded through W_out).

Per-core dataflow (all PE matmuls bf16, f32 PSUM accumulate):
  xT    = x_b^T                             transposed + chunk-major packed on
                                            HOST so DMA runs are 8KB/partition
  qkT   = (Wqk xT) + b_qk                   feature-major [e, t]
  v     = xT^T Wv^T                         token-major  [t, e_v] with ones cols
  sT_i  = kT_h[:,tk_i]^T qT_h               scoresT [tk, tq], row-packed head
                                            pairs; hh1 written base-512 so the
                                            exp input is one contiguous span
  e_i   = exp(0.125 * sT_i)                 ACT, one op per group
  mask  = tri-multiply on ex (DVE)          diag groups ordered FIRST per phase
                                            so the exp+mask chain has slack
  paU   = sum_i [v_i | 1]^T e_i             PV matmul; rows 64:128 = Z replicated
  attnT = paU[0:64] * recip(paU[64:128])    recip_approx_fast + normalize on DVE
  y     = attnT^T WoT                       partial out-projection [t, e_out]

DMA: x on the SP HWDGE ring (chunk 0 split in halves), weights on the ACT
HWDGE ring (runs in parallel), tiny tensors on POOL SWDGE.  y stores on SP.
PE stream is organized as per-group bursts [scores(g) | PV(g-2)] that share
one semaphore wait so matmuls pipeline back-to-back and HAM stays warm.
"""

import os
import sys

sys.path.insert(0, "/opt/trn_rl_repo")

import numpy as np
import ml_dtypes

import concourse.bass as bass
import concourse.mybir as mybir
from concourse.bass_utils import run_bass_kernel_spmd

dt = mybir.dt
F32, BF16 = dt.float32, dt.bfloat16
bf = ml_dtypes.bfloat16
AF = mybir.ActivationFunctionType
ALU = mybir.AluOpType

B, T, D, H = 2, 2048, 1024, 16
HD = D // H                 # 64
HPC = 4                     # heads per core
N_CORES = 8
TB = T // 128               # 16 token blocks
DB = D // 128               # 8 feature blocks of x
NC_CHUNK = 512              # tq chunk
NJ = T // NC_CHUNK          # 4 chunks
CHW = DB * NC_CHUNK         # 4096: xT cols per chunk (chunk-major layout)
SCALE = 1.0 / np.sqrt(HD)   # 0.125


class Emit:
    """Tracks semaphore counts on the python side while emitting."""

    def __init__(self, nc):
        self.nc = nc
        self.last_wait = {}

    def wge(self, eng, sem, val):
        """Standalone 1-wait instruction; skipped if this engine already
        waited for >= val on this sem."""
        if val <= 0:
            return
        key = (id(eng), id(sem))
        if self.last_wait.get(key, -1) >= val:
            return
        self.last_wait[key] = val
        eng.wait_ge(sem, val)


def build_nc(warmup=True):
    nc = bass.Bass()

    # x host-packed chunk-major: xp[p, J*4096 + db*512 + c] = x[512J+c, db*128+p]
    xt_d = nc.dram_tensor("xt", [128, NJ * CHW], BF16, kind="ExternalInput")
    # weights host-packed partition-major so DMA runs are 2-4KB:
    # wqk[eb*128+p, db*128+c], wv[p, db*256+c], wo[p, fb*1024+c]
    wqk_d = nc.dram_tensor("wqk", [512, 1024], BF16, kind="ExternalInput")
    wv_d = nc.dram_tensor("wv", [128, 2048], BF16, kind="ExternalInput")
    wo_d = nc.dram_tensor("wo", [128, 2048], BF16, kind="ExternalInput")
    bqk_d = nc.dram_tensor("bqk", [128, 4], F32, kind="ExternalInput")
    msk_d = nc.dram_tensor("msk", [128, 256], BF16, kind="ExternalInput")
    y_d = nc.dram_tensor("y", [T, D], BF16, kind="ExternalOutput")

    from contextlib import ExitStack

    ctx = ExitStack()
    sem = lambda n: ctx.enter_context(nc.semaphore(n))
    sb = lambda n, s, t: ctx.enter_context(nc.sbuf_tensor(n, s, t))
    psum = lambda n, s: ctx.enter_context(nc.psum_tensor(n, s, F32))

    sLWe = [sem(f"sLW{eb}") for eb in range(4)]   # one per wqk eb load
    sWV = sem("sWV")    # wv load +16
    sBQ = sem("sBQ")    # bqk load +16
    sMQ = sem("sMQ")    # msk load +16
    sWO = sem("sWO")    # wo load +16
    sXT = [sem(f"sXT{j}") for j in range(NJ)]     # one per xT chunk load
    sX0 = sem("sX0")    # first half (db 0-3) of x chunk 0, +16
    sMS = sem("sMS")    # vo ones memset done (POOL), +1
    sPR = sem("sPR")    # Q-proj psum groups done (PE), +1
    sYP = sem("sYP")    # O-proj psum groups done (PE), +1
    sPP = sem("sPP")    # unified pp-psum consumer count (DVE), +1
    sSC = sem("sSC")    # score groups done (PE), +1 per group
    sEX = sem("sEX")    # exp groups done (ACT), +1
    sPV = sem("sPV")    # PV matmuls done, +1
    sMK = sem("sMK")    # tri-masked diag tiles done (DVE), +1
    sNM = sem("sNM")    # normalized attnT written (DVE), +1 per (pair,J,hh)
    sST = sem("sST")    # y stores (SP HWDGE), +16
    sSTP = sem("sSTP")  # y stores (POOL SWDGE), +16 -- SWDGE sems must not
                        # be shared with HWDGE increments
    sYC = sem("sYC")    # y copies done on ACT (final drain only), +1
    sRC = sem("sRC")    # ACT 1/Z = exp(-ln Z) done, +1 per (pair,J,hh)

    xT = sb("xT", [128, NJ * CHW], BF16)              # 32KB/part, chunk-major
    wqk_s = sb("wqk_s", [128, DB * 512], BF16)        # 8KB/part
    wv_s = sb("wv_s", [128, DB * 256], BF16)          # 4KB/part
    wo_s = sb("wo_s", [128, 2 * D], BF16)             # 4KB/part
    bqk_s = sb("bqk_s", [128, 4], F32)
    msk_s = sb("msk_s", [128, 256], BF16)             # tri keep-mask | unused
    qk_s = sb("qk_s", [128, 4 * T], BF16)             # 16KB/part
    vo_s = sb("vo_s", [128, TB * 512], BF16)          # 16KB/part
    ex_s = sb("ex_s", [128, 6 * 1024], BF16)          # 6 ring slots of [128,1024]
    rz_s = sb("rz_s", [128, 512], F32)                # hh0 rows 0:64, hh1 64:128
    rl_s = sb("rl_s", [128, 512], F32)                # ln(Z) staging for 1/Z
    at_s = sb("at_s", [128, 2 * T], BF16)             # 8KB/part
    y_s = sb("y_s", [128, 16 * 1024], BF16)           # one slot per tb: no
                                                      # reuse, no receipt waits

    pp = [psum("pp0", [128, 512]), psum("pp1", [128, 512])]
    ps2 = [psum("ps2a", [128, 1024]), psum("ps2b", [128, 1024])]
    pa = [psum("pa0", [128, 512]), psum("pa1", [128, 512])]
    # 4-deep psum ring for the final (J=3) out-proj drain: the score banks,
    # which are free once the last phase's exps are done (gated by sNM)
    drain_slots = [ps2[0][:, 0:512], ps2[0][:, 512:1024],
                   ps2[1][:, 0:512], ps2[1][:, 512:1024]]

    em = Emit(nc)
    PE, ACT, DVE, SP, POOL = nc.tensor, nc.scalar, nc.vector, nc.sync, nc.gpsimd

    # warm the PE HAM clock while the first DMAs are in flight: dummy matmuls
    # on whatever is in SBUF (result never read); sized to end right when the
    # first real proj matmul's inputs land (~12us wall, cold-clock 427ns each)
    for _ in range(11 if warmup else 0):
        PE.matmul(ps2[1][:, 0:512], xT[:, 0:128], xT[:, 0:512],
                  start=True, stop=True)
    for _ in range(10 if warmup else 0):
        PE.matmul(ps2[1][:, 0:64], xT[:, 0:128], xT[:, 0:64],
                  start=True, stop=True)

    # ------------------------------------------------------------- DMA loads
    # Two parallel HWDGE rings.  The SP ring has ~2us lower first-byte
    # latency than the ACT ring, so the first-needed tensors (wqk eb0 +
    # x chunk-0 halves) go on SP; the rest of the weights on ACT.  Tiny
    # tensors + the vo ones-memset go through POOL (SWDGE, GPSIMD).
    def ld_wqk(eng, eb):
        eng.dma_start(
            wqk_s[:, eb * 1024:(eb + 1) * 1024],
            wqk_d[eb * 128:(eb + 1) * 128, :],
        ).then_inc(sLWe[eb], 16)

    # x on the SP HWDGE ring (chunk 0 split so h0 proj can start early),
    # weights on the ACT HWDGE ring in first-need order, tiny tensors +
    # the vo ones-memset on POOL (SWDGE is only fast for small transfers).
    SP.dma_start(xT[:, 0:CHW // 2], xt_d[:, 0:CHW // 2]).then_inc(sX0, 16)
    SP.dma_start(xT[:, CHW // 2:CHW], xt_d[:, CHW // 2:CHW]).then_inc(sXT[0], 16)
    for J in range(1, NJ):
        SP.dma_start(xT[:, J * CHW:(J + 1) * CHW],
                     xt_d[:, J * CHW:(J + 1) * CHW]).then_inc(sXT[J], 16)

    ld_wqk(ACT, 0)
    ld_wqk(ACT, 2)
    ACT.dma_start(wv_s[:, :], wv_d[:, :]).then_inc(sWV, 16)
    ld_wqk(ACT, 1)
    ld_wqk(ACT, 3)
    ACT.dma_start(wo_s[:, :], wo_d[:, :]).then_inc(sWO, 16)

    POOL.dma_start(bqk_s[:, :], bqk_d[:, :]).then_inc(sBQ, 16)
    POOL.dma_start(msk_s[:, :], msk_d[:, :]).then_inc(sMQ, 16)
    # ones columns of vo (the Z-sum trick); the DVE v-copies touch disjoint
    # columns so only PV readers wait on it
    POOL.memset(
        vo_s[:, :].rearrange("p (g e) -> p g e", e=128)[:, :, 64:128], 1.0
    ).then_inc(sMS, 1)

    # ------------------------------------------------- emission helper state
    n = dict(pr=0, yp=0, ppu=0, sc=0, ex=0, pv=0, nm=0, st=0, mk=0,
             yc=0, rc=0, ppu_copies=0)
    copy_sem = {}           # ppu index -> (sem, count) of its psum-free copy
    gidx = [0]              # global score/exp group position index
    ex_after_G = {}         # global position -> sEX count after its exp
    pv_after_G = {}         # global position -> sPV count after its PV
    qk_done = {}            # (pair, J) -> sPP after its two qk-proj copies
    v_done = {}             # tb -> sPP after its v copy
    nm_after = {}           # (pair, J) -> sNM count
    nm_prev = [0, 0]        # sNM count freeing pa[hh]

    fill_q = []             # pending filler closures (each = PE half-group)

    pend = {}               # state shared between the two halves of a group

    def xcol(J, db, off):
        """xT column index in chunk-major layout."""
        return J * CHW + db * 512 + off

    def emit_q_half(J, kind, idx, half):
        """Half of a projection psum group: 4 PE MMs; 2nd half adds DVE copy."""
        if half == 0:
            pend[(kind, idx)] = n["ppu"]
            n["ppu"] += 1
            u = pend[(kind, idx)]
            if kind == "qk":
                em.wge(PE, sLWe[idx], 16)
            else:
                em.wge(PE, sWV, 16)
            if J == 0 and half == 0:
                em.wge(PE, sX0, 16)
            else:
                em.wge(PE, sXT[J], 16)
            em.wge(PE, sPP, u - 1)
        else:
            em.wge(PE, sXT[J], 16)
        u = pend[(kind, idx)]
        dbs = range(4) if half == 0 else range(4, 8)
        if kind == "qk":
            eb = idx
            for db in dbs:
                mm = PE.matmul(
                    pp[u % 2][:, :],
                    wqk_s[:, eb * 1024 + db * 128: eb * 1024 + (db + 1) * 128],
                    xT[:, xcol(J, db, 0): xcol(J, db, 512)],
                    start=(db == 0), stop=(db == DB - 1))
        else:
            tb = idx
            Jt, r = tb // 4, tb % 4
            for db in dbs:
                mm = PE.matmul(
                    pp[u % 2][:, 0:256],
                    xT[:, xcol(Jt, db, r * 128): xcol(Jt, db, (r + 1) * 128)],
                    wv_s[:, db * 256:(db + 1) * 256],
                    start=(db == 0), stop=(db == DB - 1))
        if half == 0:
            return
        mm.then_inc(sPR, 1)
        n["pr"] += 1
        # DVE consumer
        em.wge(DVE, sPR, n["pr"])
        if kind == "qk":
            eb = idx
            em.wge(DVE, sBQ, 16)
            DVE.tensor_scalar(
                qk_s[:, eb * T + J * 512: eb * T + (J + 1) * 512],
                pp[u % 2][:, :], bqk_s[:, eb:eb + 1], None,
                op0=ALU.add).then_inc(sPP, 1)
        else:
            tb = idx
            DVE.tensor_copy(
                vo_s[:, tb * 512:(tb + 1) * 512]
                .rearrange("p (h e) -> p h e", h=4)[:, :, 0:64],
                pp[u % 2][:, 0:256].rearrange("p (h e) -> p h e", h=4),
            ).then_inc(sPP, 1)
        if kind == "qk":
            qk_left[J].discard(idx)
            for pair in range(2):
                if (pair, J) not in qk_done and \
                        not qk_left[J] & {pair, 2 + pair}:
                    qk_done[(pair, J)] = n["ppu_copies"] + 1
        else:
            v_done[idx] = n["ppu_copies"] + 1
        n["ppu_copies"] += 1
        copy_sem[u] = (sPP, n["ppu_copies"])

    qk_left = {}

    def emit_o_group(J, tb, ec, nm_need):
        yg = n["yp"]
        if J == 3:
            # final drain: 4-deep psum ring (score banks free behind sNM)
            slot = drain_slots[yg3[0] % 4]
            prev = drain_copy.get(yg3[0] - 4)
            yg3[0] += 1
        else:
            u = n["ppu"]
            n["ppu"] += 1
            slot = pp[u % 2][:, :]
            prev = copy_sem.get(u - 2)
        em.wge(PE, sWO, 16)
        em.wge(PE, sNM, nm_need)
        if prev is not None:
            em.wge(PE, prev[0], prev[1])
        elif J != 3:
            em.wge(PE, sPP, u - 1)
        for fb in range(2):
            mm = PE.matmul(
                slot,
                at_s[:, fb * T + tb * 128: fb * T + (tb + 1) * 128],
                wo_s[:, fb * D + ec * 512: fb * D + (ec + 1) * 512],
                start=(fb == 0), stop=(fb == 1))
        mm.then_inc(sYP, 1)
        n["yp"] += 1
        # y copy: final-drain groups (J==3) alternate DVE/ACT so the copies
        # don't serialize on one engine (ACT is idle after the last exp)
        ysl = y_s[:, tb * 1024 + ec * 512: tb * 1024 + ec * 512 + 512]
        if J == 3 and yg % 2 == 1:
            em.wge(ACT, sYP, n["yp"])
            ACT.activation(ysl, slot, AF.Copy,
                           scale=1.0).then_inc(sYC, 1)
            n["yc"] += 1
            csem = (sYC, n["yc"])
        else:
            em.wge(DVE, sYP, n["yp"])
            DVE.tensor_copy(ysl, slot).then_inc(sPP, 1)
            n["ppu_copies"] += 1
            csem = (sPP, n["ppu_copies"])
        if J == 3:
            drain_copy[yg3[0] - 1] = csem
        else:
            copy_sem[u] = csem
        if ec == 0:
            ec0_csem[tb] = csem
            return
        # one batched [128, 1024] store per tb (contiguous rows of y_d);
        # final-drain stores alternate SP/POOL so descriptor generation
        # for consecutive stores overlaps
        pool_store = (J == 3 and tb % 2 == 1)
        eng = POOL if pool_store else SP
        em.wge(eng, ec0_csem[tb][0], ec0_csem[tb][1])
        em.wge(eng, csem[0], csem[1])
        eng.dma_start(
            y_d[tb * 128:(tb + 1) * 128, :],
            y_s[:, tb * 1024: tb * 1024 + 1024],
        ).then_inc(sSTP if pool_store else sST, 16)
        n["st"] += 1

    yg3 = [0]
    drain_copy = {}
    ec0_csem = {}

    def pop_fill(k=1):
        for _ in range(k):
            if fill_q:
                fill_q.pop(0)[1]()

    # ------------------------------------------------------- attention loops
    def a_phase(pair, J):
        qb, kb = pair, 2 + pair
        nG = 4 * J + 4          # one group per tk-block P
        g0 = gidx[0]
        nm_loop_start = n["nm"]
        # diag-first ordering: the 4 diagonal blocks (which need exp + DVE
        # mask before their PV) go first so their chains overlap the wide
        # off-diagonal score/PV streams that follow
        seq = list(range(4 * J, 4 * J + 4)) + list(range(0, 4 * J))
        dval = lambda P: max(0, 128 * P - 512 * J)
        mk_after = {}           # position i -> sMK count after masks of seq[i]

        def pe_scores_wrap(i):
            P = seq[i]
            d = dval(P)
            for hh in range(2):
                ob = d if hh == 0 else 512
                mm = PE.matmul(
                    ps2[i % 2][:, ob: ob + 512 - d],
                    qk_s[hh * 64:(hh + 1) * 64,
                         kb * T + P * 128: kb * T + (P + 1) * 128],
                    qk_s[hh * 64:(hh + 1) * 64,
                         qb * T + J * 512 + d: qb * T + (J + 1) * 512],
                    start=True, stop=True, tile_position=(hh * 64, 0),
                )
            mm.then_inc(sSC, 1)
            n["sc"] += 1

        def pe_pv_waits(i):
            P = seq[i]
            em.wge(PE, sMS, 1)
            if P >= 4 * J:
                em.wge(PE, sMK, mk_after[i])
            else:
                em.wge(PE, sEX, ex_after_G[g0 + i])
            em.wge(PE, sPP, v_done[P])
            if i == 0:
                em.wge(PE, sNM, max(nm_prev))

        def pe_pv(i):
            P = seq[i]
            d = dval(P)
            slot = ((g0 + i) % 6) * 1024
            for hh in range(2):
                h = 2 * pair + hh
                eb = (slot + d) if hh == 0 else (slot + 512)
                mm = PE.matmul(
                    pa[hh][:, d:512],
                    vo_s[:, P * 512 + h * 128: P * 512 + (h + 1) * 128],
                    ex_s[:, eb: eb + 512 - d],
                    start=(i == 0), stop=(i == nG - 1),
                )
                mm.then_inc(sPV, 1)
                n["pv"] += 1
            pv_after_G[g0 + i] = n["pv"]

        for i in range(nG):
            P = seq[i]
            # ensure the v block for the PV in this burst is scheduled
            if i >= 2:
                while seq[i - 2] not in v_done:
                    assert fill_q
                    pop_fill()
            # burst waits hoisted up front (one set), then all four MMs run
            # back-to-back so LDWEIGHTS prefetch pipelines across them
            em.wge(PE, sPP, qk_done[(pair, J)])
            em.wge(PE, sEX, ex_after_G.get(g0 + i - 2, 0))
            if i >= 2:
                pe_pv_waits(i - 2)
            pe_scores_wrap(i)
            if i >= 2:
                pe_pv(i - 2)
            # keep a few fillers in reserve for the late phase boundaries
            if len(fill_q) > 6 or (J == 3 and pair == 1 and len(fill_q) > 3):
                pop_fill()
            # ACT exp for group at position i: one contiguous span [d, 1024-d)
            d = dval(P)
            slot = ((g0 + i) % 6) * 1024
            em.wge(ACT, sSC, n["sc"])
            em.wge(ACT, sPV, pv_after_G.get(g0 + i - 6, 0))
            ACT.activation(
                ex_s[:, slot + d: slot + 1024 - d],
                ps2[i % 2][:, d:1024 - d], AF.Exp, scale=float(SCALE),
            ).then_inc(sEX, 1)
            n["ex"] += 1
            ex_after_G[g0 + i] = n["ex"]
            if P >= 4 * J:
                for hh in range(2):
                    mb = (slot + d) if hh == 0 else (slot + 512)
                    em.wge(DVE, sMQ, 16)
                    em.wge(DVE, sEX, ex_after_G[g0 + i])
                    DVE.tensor_tensor(
                        ex_s[:, mb: mb + 128], ex_s[:, mb: mb + 128],
                        msk_s[:, 0:128], op=ALU.mult).then_inc(sMK, 1)
                    n["mk"] += 1
                mk_after[i] = n["mk"]
        for i in range(max(0, nG - 2), nG):
            while seq[i] not in v_done:
                assert fill_q
                pop_fill()
            pe_pv_waits(i)
            pe_pv(i)
        # pop fillers BEFORE the recip/stt emission: their PE MMs cover the
        # ACT recip latency, and their DVE copies enter the queue ahead of
        # the sRC-blocked stts (else proj copies stall the next phase)
        pop_fill(3)
        gidx[0] += nG

        # tail: 1/Z = exp(-ln Z) on ACT -- Ln and Exp share one table set
        # (natural_log_exp_and_others) so there are no table switches; then
        # DVE normalizes.  Interleaved per hh to shorten the chain into the
        # next phase's PV start.
        fb = pair
        rc_at = {}
        em.wge(ACT, sNM, nm_loop_start)   # rz_s/rl_s free of prev loop's stt
        for hh in range(2):
            em.wge(ACT, sPV, n["pv"] - (1 if hh == 0 else 0))
            ACT.activation(rl_s[hh * 64:(hh + 1) * 64, :],
                           pa[hh][64:128, :], AF.Ln,
                           scale=1.0).then_inc(sRC, 1)
            n["rc"] += 1
            em.wge(ACT, sRC, n["rc"])
            ACT.activation(rz_s[hh * 64:(hh + 1) * 64, :],
                           rl_s[hh * 64:(hh + 1) * 64, :], AF.Exp,
                           scale=-1.0).then_inc(sRC, 1)
            n["rc"] += 1
            rc_at[hh] = n["rc"]
        for hh in range(2):
            em.wge(DVE, sRC, rc_at[hh])
            DVE.scalar_tensor_tensor(
                at_s[hh * 64:(hh + 1) * 64,
                     fb * T + J * 512: fb * T + (J + 1) * 512],
                pa[hh][0:64, :], 1.0, rz_s[hh * 64:(hh + 1) * 64, :],
                op0=ALU.mult, op1=ALU.mult).then_inc(sNM, 1)
            n["nm"] += 1
            nm_prev[hh] = n["nm"]
        nm_after[(pair, J)] = n["nm"]

    # -------------------------------------------------------------- schedule
    def push_q_chunk(J):
        qk_left[J] = {0, 1, 2, 3}
        if J == 0:
            # h0 pieces only need the first x half (sX0) -- run them first so
            # the PE starts as soon as POOL lands eb0+chunk0a
            hu = [("qk", 0, 0), ("qk", 2, 0), ("qk", 0, 1), ("qk", 2, 1),
                  ("v", 0, 0), ("v", 0, 1), ("v", 1, 0), ("v", 1, 1),
                  ("qk", 1, 0), ("qk", 1, 1), ("qk", 3, 0), ("qk", 3, 1),
                  ("v", 2, 0), ("v", 2, 1), ("v", 3, 0), ("v", 3, 1)]
        else:
            units = [("qk", 0), ("qk", 2), ("v", 4 * J), ("v", 4 * J + 1),
                     ("qk", 1), ("qk", 3), ("v", 4 * J + 2), ("v", 4 * J + 3)]
            hu = [(k, x, h) for k, x in units for h in range(2)]
        for kind, idx, half in hu:
            fill_q.append((("chunk", J),
                lambda J=J, k=kind, x=idx, h=half: emit_q_half(J, k, x, h)))

    def drain_until_qk(J):
        while (0, J) not in qk_done:
            pop_fill()

    for J in range(NJ):
        push_q_chunk(J)

    for J in range(NJ):
        drain_until_qk(J)
        for pair in range(2):
            a_phase(pair, J)
        nm_need = nm_after[(1, J)]
        # out-proj groups for J are runnable once the next chunk's proj is
        # done -- insert them after chunk J+1's closures so the inter-phase
        # drain never blocks on the not-yet-ready sNM wait
        out_closures = []
        for tbl in range(4):
            tb = 4 * J + tbl
            for ec in range(2):
                out_closures.append((("out", J),
                    lambda J=J, tb=tb, ec=ec, nm_need=nm_need:
                    emit_o_group(J, tb, ec, nm_need)))
        # interleave out-groups into the queue AFTER chunk J+1's closures
        # (bunched out-groups stall PE on the DVE y-copy via the pp banks;
        # out-groups in the inter-phase drain stall on the fresh sNM).
        # For J==3 the leftovers go FIRST: they are immediately runnable and
        # cover the tail recip chain, while the out groups block on sNM.
        rest = list(fill_q)
        fill_q.clear()
        if J == 3:
            fill_q.extend(rest)
            fill_q.extend(out_closures)
        else:
            head = [c for c in rest if c[0] == ("chunk", J + 1)]
            rest = [c for c in rest if c[0] != ("chunk", J + 1)]
            merged = list(head)
            while rest or out_closures:
                if out_closures:
                    merged.append(out_closures.pop(0))
                merged.extend(rest[:2])
                rest = rest[2:]
            fill_q.extend(merged)
    while fill_q:
        pop_fill()

    ctx.close()
    return nc


_NC_CACHE = None


def _get_nc():
    global _NC_CACHE
    if _NC_CACHE is None:
        _NC_CACHE = build_nc()
    return _NC_CACHE


def _prep_in_maps(x, W_qkv, b_qkv, W_out, b_out):
    tri = np.triu(np.ones((128, 128), np.float32))          # keep p<=f
    msk = np.concatenate([tri, np.zeros((128, 128))], axis=1).astype(bf)
    in_maps = []
    for core in range(N_CORES):
        b = core // 4
        heads = [4 * (core % 4) + j for j in range(HPC)]
        fcols = np.concatenate([np.arange(h * HD, (h + 1) * HD) for h in heads])
        # reference packs W_qkv rows per head: head h = rows h*3hd + [q|k|v]
        qrows = np.concatenate(
            [np.arange(h * 3 * HD, h * 3 * HD + HD) for h in heads])
        krows = qrows + HD
        vrows = qrows + 2 * HD
        wqk = W_qkv[np.concatenate([qrows, krows]), :].T             # [D, 512]
        wv = W_qkv[vrows, :].T                                       # [D, 256]
        wo = W_out[:, fcols].T                                       # [256, D]
        # pack partition-major for long DMA runs (see dram tensor comments)
        wqk = np.ascontiguousarray(
            wqk.reshape(8, 128, 4, 128).transpose(2, 1, 0, 3)
            .reshape(512, 1024)).astype(bf)
        wv = np.ascontiguousarray(
            wv.reshape(8, 128, 256).transpose(1, 0, 2)
            .reshape(128, 2048)).astype(bf)
        wo = np.ascontiguousarray(
            wo.reshape(2, 128, 1024).transpose(1, 0, 2)
            .reshape(128, 2048)).astype(bf)
        bqk = np.ascontiguousarray(
            b_qkv[np.concatenate([qrows, krows])].reshape(4, 128).T
        ).astype(np.float32)                                          # [128, 4]
        # x chunk-major: xp[p, J*4096 + db*512 + c] = x[b][512J+c, db*128+p]
        xt = x[b].T                                                   # [D, T]
        xp = np.ascontiguousarray(
            xt.reshape(8, 128, 4, 512).transpose(1, 2, 0, 3)
            .reshape(128, NJ * CHW)).astype(bf)
        in_maps.append({
            "xt": xp,
            "wqk": wqk, "wv": wv, "wo": wo, "bqk": bqk, "msk": msk,
        })
    return in_maps


def _gather(results, b_qkv, W_out, b_out):
    out = np.zeros((B, T, D), np.float32)
    for core in range(N_CORES):
        out[core // 4] += np.asarray(results[core]["y"], np.float32)
    vidx = np.concatenate(
        [np.arange(h * 3 * HD + 2 * HD, (h + 1) * 3 * HD) for h in range(H)])
    b_v = b_qkv[vidx]
    out += b_out + b_v @ W_out.T
    return out


def _as_f32(*arrs):
    return [np.asarray(a, np.float32) for a in arrs]


def kernel(x, W_qkv, b_qkv, W_out, b_out):
    x, W_qkv, b_qkv, W_out, b_out = _as_f32(x, W_qkv, b_qkv, W_out, b_out)
    in_maps = _prep_in_maps(x, W_qkv, b_qkv, W_out, b_out)
    res = run_bass_kernel_spmd(_get_nc(), in_maps,
                               core_ids=list(range(N_CORES)))
    return _gather(res.results, b_qkv, W_out, b_out)


def run_traced(inputs, trace_cores=None):
    x, W_qkv, b_qkv, W_out, b_out = _as_f32(
        inputs["x"], inputs["W_qkv"], inputs["b_qkv"],
        inputs["W_out"], inputs["b_out"])
    in_maps = _prep_in_maps(x, W_qkv, b_qkv, W_out, b_out)
    res = run_bass_kernel_spmd(_get_nc(), in_maps,
                               core_ids=list(range(N_CORES)),
                               trace=True, trace_cores=trace_cores)
    res.gathered = _gather(res.results, b_qkv, W_out, b_out)
    return res
